# revision 21
# baseline (speedup 1.0000x reference)
"""Multi-head attention + residual + layernorm Trainium2 kernel.

Sharding: 8 cores = 4 batches x 2 query-halves (data parallel, no
collectives). Each core computes full MHA for 1024 query rows of one
batch against that batch's (mask-compacted) keys/values.

Device pipeline per core:
  - QKV projections (bf16 matmuls, fp32 PSUM accumulation)
  - per head: scoresT[keys, queries] = khT.T @ qhT
  - exp fused with mask bias + 1/sqrt(dk) scale on ScalarE
  - attT accumulation with ones-column -> softmax denominators for free
  - normalize via reciprocal + partition-broadcast
  - out projection, residual add (fp32 q), LayerNorm (bn_stats/bn_aggr)
"""

import math
from contextlib import ExitStack

import ml_dtypes
import numpy as np

import concourse.bass as bass
import concourse.mybir as mybir
import concourse.tile as tile
from concourse import bacc
from concourse.bass_utils import run_bass_kernel_spmd

# problem constants (hardcoded per harness contract)
B = 4
LQ = 2048
LK = 2048
D = 512
H = 8
DK = 64
NCORES = 8
LQC = LQ // 2  # queries per core
EPS = 1e-6
SCALE = 1.0 / math.sqrt(DK)
MASK_NEG = -30000.0

F32 = mybir.dt.float32
BF16 = mybir.dt.bfloat16
NPBF = ml_dtypes.bfloat16

AF = mybir.ActivationFunctionType
ALU = mybir.AluOpType


def build_core_program(nc, LKP):
    """Emit the single-core SPMD program. LKP = padded compacted key count."""
    NKB = LKP // 128
    kchunks = [(s, min(512, LKP - s)) for s in range(0, LKP, 512)]

    # ---- DRAM parameters (per-core shards supplied via in_maps) ----
    qT_d = nc.declare_dram_parameter("qT", [D, LQC], BF16, isOutput=False)
    qres_d = nc.declare_dram_parameter("qres", [LQC, D], F32, isOutput=False)
    kT_d = nc.declare_dram_parameter("kT", [D, LKP], BF16, isOutput=False)
    vT_d = nc.declare_dram_parameter("vT", [D, LKP], BF16, isOutput=False)
    mb_d = nc.declare_dram_parameter("mb", [128, NKB], F32, isOutput=False)
    WqT_d = nc.declare_dram_parameter("WqT", [D, D], BF16, isOutput=False)
    WkT_d = nc.declare_dram_parameter("WkT", [D, D], BF16, isOutput=False)
    WvT_d = nc.declare_dram_parameter("WvT", [D, D], BF16, isOutput=False)
    WoT_d = nc.declare_dram_parameter("WoT", [D, D], BF16, isOutput=False)
    bq_d = nc.declare_dram_parameter("bq", [128, 4], F32, isOutput=False)
    bk_d = nc.declare_dram_parameter("bk", [128, 4], F32, isOutput=False)
    bv_d = nc.declare_dram_parameter("bv", [D], F32, isOutput=False)
    gamma_d = nc.declare_dram_parameter("gamma", [D], F32, isOutput=False)
    beta_d = nc.declare_dram_parameter("beta", [D], F32, isOutput=False)
    out_d = nc.declare_dram_parameter("out", [LQC, D], F32, isOutput=True)

    with ExitStack() as ctx:
        tc = ctx.enter_context(tile.TileContext(nc))

        const = ctx.enter_context(tc.tile_pool(name="const", bufs=1))
        persist = ctx.enter_context(tc.tile_pool(name="persist", bufs=1))
        work = ctx.enter_context(tc.tile_pool(name="work", bufs=3))
        epi = ctx.enter_context(tc.tile_pool(name="epi", bufs=2))
        psA = ctx.enter_context(tc.tile_pool(name="psA", bufs=2, space="PSUM"))
        psB = ctx.enter_context(tc.tile_pool(name="psB", bufs=4, space="PSUM"))
        dramp = ctx.enter_context(tc.tile_pool(name="dramp", bufs=4, space="DRAM"))

        # ---- constants ----
        eps_s = const.tile([128, 1], F32)
        nc.vector.memset(eps_s, EPS)
        mb_s = const.tile([128, NKB], F32)
        nc.sync.dma_start(mb_s, mb_d[:, :])
        bq_s = const.tile([128, 4], F32)
        nc.sync.dma_start(bq_s, bq_d[:, :])
        bk_s = const.tile([128, 4], F32)
        nc.sync.dma_start(bk_s, bk_d[:, :])

        def _pbcast(handle):
            ap = handle.ap()
            return bass.AP(tensor=ap.tensor, offset=ap.offset,
                           ap=[[0, 128]] + list(ap.ap))

        bv_s = const.tile([128, D], F32)
        nc.sync.dma_start(bv_s, _pbcast(bv_d))
        gamma_s = const.tile([128, D], F32)
        nc.sync.dma_start(gamma_s, _pbcast(gamma_d))
        beta_s = const.tile([128, D], F32)
        nc.sync.dma_start(beta_s, _pbcast(beta_d))

        # ---- persistent intermediates ----
        qhT_s = persist.tile([128, 4, LQC], BF16)    # [e%128, e//128, q]
        khT_s = persist.tile([128, 4, LKP], BF16)    # [e%128, e//128, k]
        vh_s = persist.tile([128, NKB, H, 65], BF16)  # [k%128, kb, h, dv+1]
        attT_s = persist.tile([128, 4, LQC], BF16)   # [e%128, e//128, q]
        Wo_s = persist.tile([128, 4, D], BF16)       # [e%128, e//128, d]

        nc.sync.dma_start(Wo_s, WoT_d.ap().rearrange("(c p) d -> p c d", p=128))

        # ones column of vh (softmax denominator trick)
        nc.vector.memset(vh_s[:, :, :, 64:65], 1.0)

        # =========== phase 1: projections ===========
        with tc.tile_pool(name="projin", bufs=1) as projin:
            qT_s = projin.tile([128, 4, LQC], BF16)
            nc.sync.dma_start(qT_s, qT_d.ap().rearrange("(c p) q -> p c q", p=128))
            kT_s = projin.tile([128, 4, LKP], BF16)
            nc.sync.dma_start(kT_s, kT_d.ap().rearrange("(c p) k -> p c k", p=128))
            vT_s = projin.tile([128, 4, LKP], BF16)
            nc.sync.dma_start(vT_s, vT_d.ap().rearrange("(c p) k -> p c k", p=128))
            Wq_s = projin.tile([128, 4, D], BF16)
            nc.sync.dma_start(Wq_s, WqT_d.ap().rearrange("(c p) e -> p c e", p=128))
            Wk_s = projin.tile([128, 4, D], BF16)
            nc.sync.dma_start(Wk_s, WkT_d.ap().rearrange("(c p) e -> p c e", p=128))
            Wv_s = projin.tile([128, 4, D], BF16)
            nc.sync.dma_start(Wv_s, WvT_d.ap().rearrange("(c p) e -> p c e", p=128))

            # qhT[e, q] += WqT[dm, e].T @ qT[dm, q]
            for ec in range(4):
                ps = psA.tile([128, LQC], F32, tag="ps", name=f"psq_{ec}")
                for kc in range(4):
                    for qc in range(2):
                        nc.tensor.matmul(
                            ps[:, qc * 512:(qc + 1) * 512],
                            lhsT=Wq_s[:, kc, ec * 128:(ec + 1) * 128],
                            rhs=qT_s[:, kc, qc * 512:(qc + 1) * 512],
                            start=(kc == 0), stop=(kc == 3),
                        )
                nc.vector.tensor_scalar_add(qhT_s[:, ec, :], ps, bq_s[:, ec:ec + 1])

            # khT[e, k]
            for ec in range(4):
                for ci, (s, w) in enumerate(kchunks):
                    ps = psA.tile([128, LQC], F32, tag="ps", name=f"psk_{ec}_{ci}")
                    for kc in range(4):
                        nc.tensor.matmul(
                            ps[:, 0:w],
                            lhsT=Wk_s[:, kc, ec * 128:(ec + 1) * 128],
                            rhs=kT_s[:, kc, s:s + w],
                            start=(kc == 0), stop=(kc == 3),
                        )
                    nc.vector.tensor_scalar_add(
                        khT_s[:, ec, s:s + w], ps[:, 0:w], bk_s[:, ec:ec + 1]
                    )

            # vh[k, e] natural layout in 65-wide head groups (+bv on copy-out)
            for kb in range(NKB):
                ps = psB.tile([128, 512], F32, tag="att", name=f"psv_{kb}")
                for kc in range(4):
                    nc.tensor.matmul(
                        ps,
                        lhsT=vT_s[:, kc, kb * 128:(kb + 1) * 128],
                        rhs=Wv_s[:, kc, :],
                        start=(kc == 0), stop=(kc == 3),
                    )
                nc.vector.scalar_tensor_tensor(
                    vh_s[:, kb, :, 0:64],
                    ps.rearrange("p (h e) -> p h e", h=H),
                    1.0,
                    bv_s.rearrange("p (h e) -> p h e", h=H),
                    op0=ALU.mult, op1=ALU.add,
                )

        # =========== phase 2: attention ===========
        for hp in range(4):  # head pairs (h = 2*hp + hi); hi selects partition half
            att_ps = {}
            for hi in range(2):
                for qc in range(2):
                    att_ps[(hi, qc)] = psB.tile([65, 512], F32, tag="att",
                                                name=f"att_{hp}_{hi}_{qc}")
            for kb in range(NKB):
                exp_t = {}
                for hi in range(2):
                    p0 = 64 * hi
                    sc = psA.tile([128, LQC], F32, tag="ps", name=f"sc_{hp}_{hi}_{kb}")
                    for qc in range(2):
                        nc.tensor.matmul(
                            sc[:, qc * 512:(qc + 1) * 512],
                            lhsT=khT_s[p0:p0 + 64, hp, kb * 128:(kb + 1) * 128],
                            rhs=qhT_s[p0:p0 + 64, hp, qc * 512:(qc + 1) * 512],
                            start=True, stop=True,
                        )
                    et = work.tile([128, LQC], BF16, tag="exp",
                                   name=f"exp_{hp}_{hi}_{kb}")
                    nc.scalar.activation(et, sc, AF.Exp,
                                         bias=mb_s[:, kb:kb + 1], scale=SCALE)
                    exp_t[hi] = et
                for hi in range(2):
                    h = 2 * hp + hi
                    for qc in range(2):
                        nc.tensor.matmul(
                            att_ps[(hi, qc)],
                            lhsT=vh_s[:, kb, h, :],
                            rhs=exp_t[hi][:, qc * 512:(qc + 1) * 512],
                            start=(kb == 0), stop=(kb == NKB - 1),
                        )
            for hi in range(2):
                p0 = 64 * hi
                for qc in range(2):
                    ap_t = att_ps[(hi, qc)]
                    rc = work.tile([1, 512], F32, tag="recip",
                                   name=f"rc_{hp}_{hi}_{qc}")
                    nc.vector.reciprocal(rc, ap_t[64:65, :])
                    # partition-broadcast via DRAM bounce (SBUF APs cannot
                    # have zero partition step; DRAM APs can)
                    rc_d = dramp.tile([1, 512], F32, tag="rcd",
                                      name=f"rcd_{hp}_{hi}_{qc}")
                    nc.sync.dma_start(rc_d, rc)
                    rd_ap = rc_d[0:1, :]
                    rc_b = bass.AP(tensor=rd_ap.tensor, offset=rd_ap.offset,
                                   ap=[[0, 64]] + list(rd_ap.ap)[1:])
                    Rt = work.tile([64, 512], F32, tag="R",
                                   name=f"R_{hp}_{hi}_{qc}")
                    nc.sync.dma_start(Rt, rc_b)
                    nc.vector.tensor_tensor(
                        attT_s[p0:p0 + 64, hp, qc * 512:(qc + 1) * 512],
                        ap_t[0:64, :], Rt, op=ALU.mult,
                    )

        # =========== phase 3: out projection + residual + layernorm ===========
        for qb in range(8):
            qres_t = epi.tile([128, D], F32, tag="qres", name=f"qres_{qb}")
            nc.sync.dma_start(
                qres_t, qres_d[qb * 128:(qb + 1) * 128, :])
            po = psB.tile([128, 512], F32, tag="att", name=f"po_{qb}")
            for ec in range(4):
                nc.tensor.matmul(
                    po,
                    lhsT=attT_s[:, ec, qb * 128:(qb + 1) * 128],
                    rhs=Wo_s[:, ec, :],
                    start=(ec == 0), stop=(ec == 3),
                )
            x = epi.tile([128, D], F32, tag="x", name=f"x_{qb}")
            nc.vector.tensor_add(x, po, qres_t)
            st6 = epi.tile([128, 6], F32, tag="st6", name=f"st6_{qb}")
            nc.vector.bn_stats(st6, x)
            mv = epi.tile([128, 2], F32, tag="mv", name=f"mv_{qb}")
            nc.vector.bn_aggr(mv, st6)
            sd = epi.tile([128, 1], F32, tag="sd", name=f"sd_{qb}")
            nc.scalar.activation(sd, mv[:, 1:2], AF.Sqrt, bias=eps_s[:, 0:1],
                                 scale=1.0)
            rs = epi.tile([128, 1], F32, tag="rs", name=f"rs_{qb}")
            nc.vector.reciprocal(rs, sd)
            y = epi.tile([128, D], F32, tag="y", name=f"y_{qb}")
            nc.vector.tensor_scalar(y, x, mv[:, 0:1], rs,
                                    op0=ALU.subtract, op1=ALU.mult)
            y2 = epi.tile([128, D], F32, tag="qres", name=f"y2_{qb}")
            nc.vector.tensor_tensor(y2, y, gamma_s, op=ALU.mult)
            y3 = epi.tile([128, D], F32, tag="x", name=f"y3_{qb}")
            nc.vector.tensor_tensor(y3, y2, beta_s, op=ALU.add)
            nc.sync.dma_start(out_d[qb * 128:(qb + 1) * 128, :], y3)

    return nc


def prepare_host_inputs(q, k, v, att_mask, Wq, bq, Wk, bk, Wv, bv, Wo, bo,
                        gamma, beta):
    """Host-side sharding: mask compaction, transposes, per-core in_maps."""
    q = np.asarray(q, np.float32)
    k = np.asarray(k, np.float32)
    v = np.asarray(v, np.float32)
    att_mask = np.asarray(att_mask)
    valid = ~att_mask  # True = keep
    counts = valid.sum(axis=1)
    LKP = int(max(128, ((int(counts.max()) + 127) // 128) * 128))
    NKB = LKP // 128

    kT_p = np.zeros((B, D, LKP), NPBF)
    vT_p = np.zeros((B, D, LKP), NPBF)
    mb = np.full((B, LKP), MASK_NEG, np.float32)
    for b in range(B):
        idx = np.nonzero(valid[b])[0]
        n = len(idx)
        kT_p[b, :, :n] = k[b, idx, :].T.astype(NPBF)
        vT_p[b, :, :n] = v[b, idx, :].T.astype(NPBF)
        mb[b, :n] = 0.0

    shared = {
        "WqT": np.ascontiguousarray(np.asarray(Wq, np.float32).T).astype(NPBF),
        "WkT": np.ascontiguousarray(np.asarray(Wk, np.float32).T).astype(NPBF),
        "WvT": np.ascontiguousarray(np.asarray(Wv, np.float32).T).astype(NPBF),
        "WoT": np.ascontiguousarray(np.asarray(Wo, np.float32).T).astype(NPBF),
        "bq": np.ascontiguousarray(np.asarray(bq, np.float32).reshape(4, 128).T),
        "bk": np.ascontiguousarray(np.asarray(bk, np.float32).reshape(4, 128).T),
        "bv": np.asarray(bv, np.float32).copy(),
        "gamma": np.asarray(gamma, np.float32).copy(),
        "beta": np.asarray(beta, np.float32).copy(),
    }
    bo_f = np.asarray(bo, np.float32)

    in_maps = []
    for core in range(NCORES):
        b, qh = divmod(core, 2)
        sl = slice(qh * LQC, (qh + 1) * LQC)
        m = dict(shared)
        m["qT"] = np.ascontiguousarray(q[b, sl, :].T).astype(NPBF)
        m["qres"] = q[b, sl, :] + bo_f  # residual with out-proj bias folded in
        m["kT"] = kT_p[b]
        m["vT"] = vT_p[b]
        m["mb"] = np.ascontiguousarray(mb[b].reshape(NKB, 128).T)
        in_maps.append(m)
    return in_maps, LKP


_program_cache = {}


def get_program(LKP):
    if LKP not in _program_cache:
        nc = bacc.Bacc("TRN2", target_bir_lowering=False, debug=False)
        build_core_program(nc, LKP)
        nc.compile()
        _program_cache[LKP] = nc
    return _program_cache[LKP]


def kernel(q, k, v, att_mask, Wq, bq, Wk, bk, Wv, bv, Wo, bo, gamma, beta,
           _trace=False, _trace_kwargs=None):
    in_maps, LKP = prepare_host_inputs(
        q, k, v, att_mask, Wq, bq, Wk, bk, Wv, bv, Wo, bo, gamma, beta)
    nc = get_program(LKP)
    res = run_bass_kernel_spmd(
        nc, in_maps, core_ids=list(range(NCORES)),
        trace=_trace, **(_trace_kwargs or {}),
    )
    out = np.empty((B, LQ, D), np.float32)
    for core in range(NCORES):
        b, qh = divmod(core, 2)
        out[b, qh * LQC:(qh + 1) * LQC, :] = res.results[core]["out"]
    kernel._last_results = res
    return out


# revision 24
# speedup vs baseline: 1.2203x; 1.2203x over previous
"""Multi-head attention + residual + layernorm Trainium2 kernel.

Sharding: 8 cores = 4 batches x 2 query-halves (data parallel, no
collectives). Each core computes full MHA for 1024 query rows of one
batch against that batch's (mask-compacted) keys/values.

Device pipeline per core:
  - QKV projections (bf16 matmuls, fp32 PSUM accumulation)
  - per head: scoresT[keys, queries] = khT.T @ qhT
  - exp fused with mask bias + 1/sqrt(dk) scale on ScalarE
  - attT accumulation with ones-column -> softmax denominators for free
  - normalize via reciprocal + partition-broadcast
  - out projection, residual add (fp32 q), LayerNorm (bn_stats/bn_aggr)
"""

import math
from contextlib import ExitStack

import ml_dtypes
import numpy as np

import concourse.bass as bass
import concourse.mybir as mybir
import concourse.tile as tile
from concourse import bacc
from concourse.bass_utils import run_bass_kernel_spmd

# problem constants (hardcoded per harness contract)
B = 4
LQ = 2048
LK = 2048
D = 512
H = 8
DK = 64
NCORES = 8
LQC = LQ // 2  # queries per core
EPS = 1e-6
SCALE = 1.0 / math.sqrt(DK)
MASK_NEG = -30000.0

F32 = mybir.dt.float32
BF16 = mybir.dt.bfloat16
NPBF = ml_dtypes.bfloat16

AF = mybir.ActivationFunctionType
ALU = mybir.AluOpType


def build_core_program(nc, LKP):
    """Emit the single-core SPMD program. LKP = padded compacted key count."""
    NKB = LKP // 128
    kchunks = [(s, min(512, LKP - s)) for s in range(0, LKP, 512)]

    # ---- DRAM parameters (per-core shards supplied via in_maps) ----
    qT_d = nc.declare_dram_parameter("qT", [D, LQC], BF16, isOutput=False)
    qres_d = nc.declare_dram_parameter("qres", [LQC, D], F32, isOutput=False)
    kT_d = nc.declare_dram_parameter("kT", [D, LKP], BF16, isOutput=False)
    vT_d = nc.declare_dram_parameter("vT", [D, LKP], BF16, isOutput=False)
    mb_d = nc.declare_dram_parameter("mb", [128, NKB], F32, isOutput=False)
    WqT_d = nc.declare_dram_parameter("WqT", [D, D], BF16, isOutput=False)
    WkT_d = nc.declare_dram_parameter("WkT", [D, D], BF16, isOutput=False)
    WvT_d = nc.declare_dram_parameter("WvT", [D, D], BF16, isOutput=False)
    WoT_d = nc.declare_dram_parameter("WoT", [D, D], BF16, isOutput=False)
    bq_d = nc.declare_dram_parameter("bq", [128, 4], F32, isOutput=False)
    bk_d = nc.declare_dram_parameter("bk", [128, 4], F32, isOutput=False)
    bv_d = nc.declare_dram_parameter("bv", [D], F32, isOutput=False)
    gamma_d = nc.declare_dram_parameter("gamma", [D], F32, isOutput=False)
    beta_d = nc.declare_dram_parameter("beta", [D], F32, isOutput=False)
    out_d = nc.declare_dram_parameter("out", [LQC, D], F32, isOutput=True)

    with ExitStack() as ctx:
        tc = ctx.enter_context(tile.TileContext(nc))

        const = ctx.enter_context(tc.tile_pool(name="const", bufs=1))
        persist = ctx.enter_context(tc.tile_pool(name="persist", bufs=1))
        work = ctx.enter_context(tc.tile_pool(name="work", bufs=3))
        epi = ctx.enter_context(tc.tile_pool(name="epi", bufs=2))
        psA = ctx.enter_context(tc.tile_pool(name="psA", bufs=2, space="PSUM"))
        psB = ctx.enter_context(tc.tile_pool(name="psB", bufs=4, space="PSUM"))
        dramp = ctx.enter_context(tc.tile_pool(name="dramp", bufs=4, space="DRAM"))

        # ---- constants ----
        eps_s = const.tile([128, 1], F32)
        nc.vector.memset(eps_s, EPS)
        mb_s = const.tile([128, NKB], F32)
        nc.sync.dma_start(mb_s, mb_d[:, :])
        bq_s = const.tile([128, 4], F32)
        nc.sync.dma_start(bq_s, bq_d[:, :])
        bk_s = const.tile([128, 4], F32)
        nc.sync.dma_start(bk_s, bk_d[:, :])

        def _pbcast(handle):
            ap = handle.ap()
            return bass.AP(tensor=ap.tensor, offset=ap.offset,
                           ap=[[0, 128]] + list(ap.ap))

        bv_s = const.tile([128, D], F32)
        nc.sync.dma_start(bv_s, _pbcast(bv_d))
        gamma_s = const.tile([128, D], F32)
        nc.sync.dma_start(gamma_s, _pbcast(gamma_d))
        beta_s = const.tile([128, D], F32)
        nc.sync.dma_start(beta_s, _pbcast(beta_d))

        # ---- persistent intermediates ----
        qhT_s = persist.tile([128, 4, LQC], BF16)    # [e%128, e//128, q]
        khT_s = persist.tile([128, 4, LKP], BF16)    # [e%128, e//128, k]
        vh_s = persist.tile([128, NKB, H, 65], BF16)  # [k%128, kb, h, dv+1]
        attT_s = persist.tile([128, 4, LQC], BF16)   # [e%128, e//128, q]
        Wo_s = persist.tile([128, 4, D], BF16)       # [e%128, e//128, d]

        nc.sync.dma_start(Wo_s, WoT_d.ap().rearrange("(c p) d -> p c d", p=128))

        # ones column of vh (softmax denominator trick)
        nc.vector.memset(vh_s[:, :, :, 64:65], 1.0)

        # =========== phase 1: projections ===========
        with tc.tile_pool(name="projin", bufs=1) as projin:
            qT_s = projin.tile([128, 4, LQC], BF16)
            nc.sync.dma_start(qT_s, qT_d.ap().rearrange("(c p) q -> p c q", p=128))
            kT_s = projin.tile([128, 4, LKP], BF16)
            nc.sync.dma_start(kT_s, kT_d.ap().rearrange("(c p) k -> p c k", p=128))
            vT_s = projin.tile([128, 4, LKP], BF16)
            nc.sync.dma_start(vT_s, vT_d.ap().rearrange("(c p) k -> p c k", p=128))
            Wq_s = projin.tile([128, 4, D], BF16)
            nc.sync.dma_start(Wq_s, WqT_d.ap().rearrange("(c p) e -> p c e", p=128))
            Wk_s = projin.tile([128, 4, D], BF16)
            nc.sync.dma_start(Wk_s, WkT_d.ap().rearrange("(c p) e -> p c e", p=128))
            Wv_s = projin.tile([128, 4, D], BF16)
            nc.sync.dma_start(Wv_s, WvT_d.ap().rearrange("(c p) e -> p c e", p=128))

            def proj_qk_pair(ec):
                """Emit qhT+khT projection for head-pair ec (interleavable)."""
                ps = psA.tile([128, LQC], F32, tag="ps", name=f"psq_{ec}")
                for kc in range(4):
                    for qc in range(2):
                        nc.tensor.matmul(
                            ps[:, qc * 512:(qc + 1) * 512],
                            lhsT=Wq_s[:, kc, ec * 128:(ec + 1) * 128],
                            rhs=qT_s[:, kc, qc * 512:(qc + 1) * 512],
                            start=(kc == 0), stop=(kc == 3),
                        )
                nc.vector.tensor_scalar_add(qhT_s[:, ec, :], ps, bq_s[:, ec:ec + 1])
                for ci, (s, w) in enumerate(kchunks):
                    ps2 = psA.tile([128, LQC], F32, tag="ps", name=f"psk_{ec}_{ci}")
                    for kc in range(4):
                        nc.tensor.matmul(
                            ps2[:, 0:w],
                            lhsT=Wk_s[:, kc, ec * 128:(ec + 1) * 128],
                            rhs=kT_s[:, kc, s:s + w],
                            start=(kc == 0), stop=(kc == 3),
                        )
                    nc.vector.tensor_scalar_add(
                        khT_s[:, ec, s:s + w], ps2[:, 0:w], bk_s[:, ec:ec + 1]
                    )

            # vh[k, e] natural layout in 65-wide head groups (+bv on copy-out)
            for kb in range(NKB):
                ps = psB.tile([128, 512], F32, tag="att", name=f"psv_{kb}")
                for kc in range(4):
                    nc.tensor.matmul(
                        ps,
                        lhsT=vT_s[:, kc, kb * 128:(kb + 1) * 128],
                        rhs=Wv_s[:, kc, :],
                        start=(kc == 0), stop=(kc == 3),
                    )
                nc.vector.scalar_tensor_tensor(
                    vh_s[:, kb, :, 0:64],
                    ps.rearrange("p (h e) -> p h e", h=H),
                    1.0,
                    bv_s.rearrange("p (h e) -> p h e", h=H),
                    op0=ALU.mult, op1=ALU.add,
                )

            # projections for head-pair 0 up front; pairs 1-3 are emitted
            # inside the attention stream so the PE can fill exp-wait gaps
            proj_qk_pair(0)

            # =========== phase 2: attention (inside projin scope for
            # interleaved projections) ===========
            for hp in range(4):
                att_ps = {}
                for hi in range(2):
                    for qc in range(2):
                        att_ps[(hi, qc)] = psB.tile([65, 512], F32, tag="att",
                                                    name=f"att_{hp}_{hi}_{qc}")
                for kb in range(NKB):
                    # interleave next pair's projections mid-stream
                    if kb == 2 and hp < 3:
                        proj_qk_pair(hp + 1)
                    sc = {}
                    for hi in range(2):
                        sc[hi] = psA.tile([128, LQC], F32, tag="ps",
                                          name=f"sc_{hp}_{hi}_{kb}")
                    # alternate row-groups (hi) so score matmuls pair up
                    # concurrently on the PE array
                    for qc in range(2):
                        for hi in range(2):
                            p0 = 64 * hi
                            nc.tensor.matmul(
                                sc[hi][:, qc * 512:(qc + 1) * 512],
                                lhsT=khT_s[p0:p0 + 64, hp,
                                           kb * 128:(kb + 1) * 128],
                                rhs=qhT_s[p0:p0 + 64, hp,
                                          qc * 512:(qc + 1) * 512],
                                start=True, stop=True,
                            )
                    exp_t = {}
                    for hi in range(2):
                        et = work.tile([128, LQC], BF16, tag="exp",
                                       name=f"exp_{hp}_{hi}_{kb}")
                        nc.scalar.activation(et, sc[hi], AF.Exp,
                                             bias=mb_s[:, kb:kb + 1],
                                             scale=SCALE)
                        exp_t[hi] = et
                    for hi in range(2):
                        h = 2 * hp + hi
                        for qc in range(2):
                            nc.tensor.matmul(
                                att_ps[(hi, qc)],
                                lhsT=vh_s[:, kb, h, :],
                                rhs=exp_t[hi][:, qc * 512:(qc + 1) * 512],
                                start=(kb == 0), stop=(kb == NKB - 1),
                            )
                for hi in range(2):
                    p0 = 64 * hi
                    for qc in range(2):
                        ap_t = att_ps[(hi, qc)]
                        rc = work.tile([1, 512], F32, tag="recip",
                                       name=f"rc_{hp}_{hi}_{qc}")
                        nc.vector.reciprocal(rc, ap_t[64:65, :])
                        # partition-broadcast via DRAM bounce (SBUF APs cannot
                        # have zero partition step; DRAM APs can)
                        rc_d = dramp.tile([1, 512], F32, tag="rcd",
                                          name=f"rcd_{hp}_{hi}_{qc}")
                        nc.sync.dma_start(rc_d, rc)
                        rd_ap = rc_d[0:1, :]
                        rc_b = bass.AP(tensor=rd_ap.tensor, offset=rd_ap.offset,
                                       ap=[[0, 64]] + list(rd_ap.ap)[1:])
                        Rt = work.tile([64, 512], F32, tag="R",
                                       name=f"R_{hp}_{hi}_{qc}")
                        nc.sync.dma_start(Rt, rc_b)
                        nc.vector.tensor_tensor(
                            attT_s[p0:p0 + 64, hp, qc * 512:(qc + 1) * 512],
                            ap_t[0:64, :], Rt, op=ALU.mult,
                        )

        # =========== phase 3: out projection + residual + layernorm ===========
        for qb in range(8):
            qres_t = epi.tile([128, D], F32, tag="qres", name=f"qres_{qb}")
            nc.sync.dma_start(
                qres_t, qres_d[qb * 128:(qb + 1) * 128, :])
            po = psB.tile([128, 512], F32, tag="att", name=f"po_{qb}")
            for ec in range(4):
                nc.tensor.matmul(
                    po,
                    lhsT=attT_s[:, ec, qb * 128:(qb + 1) * 128],
                    rhs=Wo_s[:, ec, :],
                    start=(ec == 0), stop=(ec == 3),
                )
            x = epi.tile([128, D], F32, tag="x", name=f"x_{qb}")
            nc.vector.tensor_add(x, po, qres_t)
            st6 = epi.tile([128, 6], F32, tag="st6", name=f"st6_{qb}")
            nc.vector.bn_stats(st6, x)
            mv = epi.tile([128, 2], F32, tag="mv", name=f"mv_{qb}")
            nc.vector.bn_aggr(mv, st6)
            sd = epi.tile([128, 1], F32, tag="sd", name=f"sd_{qb}")
            nc.scalar.activation(sd, mv[:, 1:2], AF.Sqrt, bias=eps_s[:, 0:1],
                                 scale=1.0)
            rs = epi.tile([128, 1], F32, tag="rs", name=f"rs_{qb}")
            nc.vector.reciprocal(rs, sd)
            y = epi.tile([128, D], F32, tag="y", name=f"y_{qb}")
            nc.vector.tensor_scalar(y, x, mv[:, 0:1], rs,
                                    op0=ALU.subtract, op1=ALU.mult)
            y2 = epi.tile([128, D], F32, tag="qres", name=f"y2_{qb}")
            nc.vector.tensor_tensor(y2, y, gamma_s, op=ALU.mult)
            y3 = epi.tile([128, D], F32, tag="x", name=f"y3_{qb}")
            nc.vector.tensor_tensor(y3, y2, beta_s, op=ALU.add)
            nc.sync.dma_start(out_d[qb * 128:(qb + 1) * 128, :], y3)

    return nc


def prepare_host_inputs(q, k, v, att_mask, Wq, bq, Wk, bk, Wv, bv, Wo, bo,
                        gamma, beta):
    """Host-side sharding: mask compaction, transposes, per-core in_maps."""
    q = np.asarray(q, np.float32)
    k = np.asarray(k, np.float32)
    v = np.asarray(v, np.float32)
    att_mask = np.asarray(att_mask)
    valid = ~att_mask  # True = keep
    counts = valid.sum(axis=1)
    LKP = int(max(128, ((int(counts.max()) + 127) // 128) * 128))
    NKB = LKP // 128

    kT_p = np.zeros((B, D, LKP), NPBF)
    vT_p = np.zeros((B, D, LKP), NPBF)
    mb = np.full((B, LKP), MASK_NEG, np.float32)
    for b in range(B):
        idx = np.nonzero(valid[b])[0]
        n = len(idx)
        kT_p[b, :, :n] = k[b, idx, :].T.astype(NPBF)
        vT_p[b, :, :n] = v[b, idx, :].T.astype(NPBF)
        mb[b, :n] = 0.0

    shared = {
        "WqT": np.ascontiguousarray(np.asarray(Wq, np.float32).T).astype(NPBF),
        "WkT": np.ascontiguousarray(np.asarray(Wk, np.float32).T).astype(NPBF),
        "WvT": np.ascontiguousarray(np.asarray(Wv, np.float32).T).astype(NPBF),
        "WoT": np.ascontiguousarray(np.asarray(Wo, np.float32).T).astype(NPBF),
        "bq": np.ascontiguousarray(np.asarray(bq, np.float32).reshape(4, 128).T),
        "bk": np.ascontiguousarray(np.asarray(bk, np.float32).reshape(4, 128).T),
        "bv": np.asarray(bv, np.float32).copy(),
        "gamma": np.asarray(gamma, np.float32).copy(),
        "beta": np.asarray(beta, np.float32).copy(),
    }
    bo_f = np.asarray(bo, np.float32)

    in_maps = []
    for core in range(NCORES):
        b, qh = divmod(core, 2)
        sl = slice(qh * LQC, (qh + 1) * LQC)
        m = dict(shared)
        m["qT"] = np.ascontiguousarray(q[b, sl, :].T).astype(NPBF)
        m["qres"] = q[b, sl, :] + bo_f  # residual with out-proj bias folded in
        m["kT"] = kT_p[b]
        m["vT"] = vT_p[b]
        m["mb"] = np.ascontiguousarray(mb[b].reshape(NKB, 128).T)
        in_maps.append(m)
    return in_maps, LKP


_program_cache = {}


def get_program(LKP):
    if LKP not in _program_cache:
        nc = bacc.Bacc("TRN2", target_bir_lowering=False, debug=False)
        build_core_program(nc, LKP)
        nc.compile()
        _program_cache[LKP] = nc
    return _program_cache[LKP]


def kernel(q, k, v, att_mask, Wq, bq, Wk, bk, Wv, bv, Wo, bo, gamma, beta,
           _trace=False, _trace_kwargs=None):
    in_maps, LKP = prepare_host_inputs(
        q, k, v, att_mask, Wq, bq, Wk, bk, Wv, bv, Wo, bo, gamma, beta)
    nc = get_program(LKP)
    res = run_bass_kernel_spmd(
        nc, in_maps, core_ids=list(range(NCORES)),
        trace=_trace, **(_trace_kwargs or {}),
    )
    out = np.empty((B, LQ, D), np.float32)
    for core in range(NCORES):
        b, qh = divmod(core, 2)
        out[b, qh * LQC:(qh + 1) * LQC, :] = res.results[core]["out"]
    kernel._last_results = res
    return out


# revision 31
# speedup vs baseline: 1.2209x; 1.0005x over previous
"""Multi-head attention + residual + layernorm Trainium2 kernel.

Sharding: 8 cores = 4 batches x 2 query-halves (data parallel, no
collectives). Each core computes full MHA for 1024 query rows of one
batch against that batch's (mask-compacted) keys/values.

Device pipeline per core:
  - QKV projections (bf16 matmuls, fp32 PSUM accumulation)
  - per head: scoresT[keys, queries] = khT.T @ qhT
  - exp fused with mask bias + 1/sqrt(dk) scale on ScalarE
  - attT accumulation with ones-column -> softmax denominators for free
  - normalize via reciprocal + partition-broadcast
  - out projection, residual add (fp32 q), LayerNorm (bn_stats/bn_aggr)
"""

import math
from contextlib import ExitStack

import ml_dtypes
import numpy as np

import concourse.bass as bass
import concourse.mybir as mybir
import concourse.tile as tile
from concourse import bacc
from concourse.bass_utils import run_bass_kernel_spmd

# problem constants (hardcoded per harness contract)
B = 4
LQ = 2048
LK = 2048
D = 512
H = 8
DK = 64
NCORES = 8
LQC = LQ // 2  # queries per core
EPS = 1e-6
SCALE = 1.0 / math.sqrt(DK)
MASK_NEG = -30000.0

F32 = mybir.dt.float32
BF16 = mybir.dt.bfloat16
NPBF = ml_dtypes.bfloat16

AF = mybir.ActivationFunctionType
ALU = mybir.AluOpType


def build_core_program(nc, LKP):
    """Emit the single-core SPMD program. LKP = padded compacted key count."""
    NKB = LKP // 128
    kchunks = [(s, min(512, LKP - s)) for s in range(0, LKP, 512)]

    # ---- DRAM parameters (per-core shards supplied via in_maps) ----
    qT_d = nc.declare_dram_parameter("qT", [D, LQC], BF16, isOutput=False)
    qres_d = nc.declare_dram_parameter("qres", [LQC, D], F32, isOutput=False)
    kT_d = nc.declare_dram_parameter("kT", [D, LKP], BF16, isOutput=False)
    vT_d = nc.declare_dram_parameter("vT", [D, LKP], BF16, isOutput=False)
    mb_d = nc.declare_dram_parameter("mb", [128, NKB], F32, isOutput=False)
    WqT_d = nc.declare_dram_parameter("WqT", [D, D], BF16, isOutput=False)
    WkT_d = nc.declare_dram_parameter("WkT", [D, D], BF16, isOutput=False)
    WvT_d = nc.declare_dram_parameter("WvT", [D, D], BF16, isOutput=False)
    WoT_d = nc.declare_dram_parameter("WoT", [D, D], BF16, isOutput=False)
    bq_d = nc.declare_dram_parameter("bq", [128, 4], F32, isOutput=False)
    bk_d = nc.declare_dram_parameter("bk", [128, 4], F32, isOutput=False)
    bv_d = nc.declare_dram_parameter("bv", [D], F32, isOutput=False)
    gamma_d = nc.declare_dram_parameter("gamma", [D], F32, isOutput=False)
    beta_d = nc.declare_dram_parameter("beta", [D], F32, isOutput=False)
    out_d = nc.declare_dram_parameter("out", [LQC, D], F32, isOutput=True)

    with ExitStack() as ctx:
        tc = ctx.enter_context(tile.TileContext(nc))

        const = ctx.enter_context(tc.tile_pool(name="const", bufs=1))
        persist = ctx.enter_context(tc.tile_pool(name="persist", bufs=1))
        work = ctx.enter_context(tc.tile_pool(name="work", bufs=3))
        epi = ctx.enter_context(tc.tile_pool(name="epi", bufs=2))
        psA = ctx.enter_context(tc.tile_pool(name="psA", bufs=2, space="PSUM"))
        psB = ctx.enter_context(tc.tile_pool(name="psB", bufs=4, space="PSUM"))
        dramp = ctx.enter_context(tc.tile_pool(name="dramp", bufs=4, space="DRAM"))

        # ---- constants ----
        eps_s = const.tile([128, 1], F32)
        nc.vector.memset(eps_s, EPS)
        mb_s = const.tile([128, NKB], F32)
        nc.sync.dma_start(mb_s, mb_d[:, :])
        bq_s = const.tile([128, 4], F32)
        nc.sync.dma_start(bq_s, bq_d[:, :])
        bk_s = const.tile([128, 4], F32)
        nc.sync.dma_start(bk_s, bk_d[:, :])

        def _pbcast(handle):
            ap = handle.ap()
            return bass.AP(tensor=ap.tensor, offset=ap.offset,
                           ap=[[0, 128]] + list(ap.ap))

        bv_s = const.tile([128, D], F32)
        nc.sync.dma_start(bv_s, _pbcast(bv_d))
        gamma_s = const.tile([128, D], F32)
        nc.sync.dma_start(gamma_s, _pbcast(gamma_d))
        beta_s = const.tile([128, D], F32)
        nc.sync.dma_start(beta_s, _pbcast(beta_d))

        # ---- persistent intermediates ----
        qhT_s = persist.tile([128, 4, LQC], BF16)    # [e%128, e//128, q]
        khT_s = persist.tile([128, 4, LKP], BF16)    # [e%128, e//128, k]
        vh_s = persist.tile([128, NKB, H, 65], BF16)  # [k%128, kb, h, dv+1]
        attT_s = persist.tile([128, 4, LQC], BF16)   # [e%128, e//128, q]
        Wo_s = persist.tile([128, 4, D], BF16)       # [e%128, e//128, d]

        nc.sync.dma_start(Wo_s, WoT_d.ap().rearrange("(c p) d -> p c d", p=128))

        # ones column of vh (softmax denominator trick)
        nc.vector.memset(vh_s[:, :, :, 64:65], 1.0)

        # =========== phase 1: projections ===========
        with tc.tile_pool(name="projin", bufs=1) as projin:
            # per-kc chunk DMAs so the first matmuls start as soon as their
            # operands land instead of after the whole input batch
            qT_s = projin.tile([128, 4, LQC], BF16)
            kT_s = projin.tile([128, 4, LKP], BF16)
            vT_s = projin.tile([128, 4, LKP], BF16)
            Wq_s = projin.tile([128, 4, D], BF16)
            Wk_s = projin.tile([128, 4, D], BF16)
            Wv_s = projin.tile([128, 4, D], BF16)
            qT_r = qT_d.ap().rearrange("(c p) q -> p c q", p=128)
            kT_r = kT_d.ap().rearrange("(c p) k -> p c k", p=128)
            vT_r = vT_d.ap().rearrange("(c p) k -> p c k", p=128)
            Wq_r = WqT_d.ap().rearrange("(c p) e -> p c e", p=128)
            Wk_r = WkT_d.ap().rearrange("(c p) e -> p c e", p=128)
            Wv_r = WvT_d.ap().rearrange("(c p) e -> p c e", p=128)
            for kc in range(4):
                nc.sync.dma_start(Wq_s[:, kc, :], Wq_r[:, kc, :])
                nc.sync.dma_start(qT_s[:, kc, :], qT_r[:, kc, :])
                nc.sync.dma_start(Wk_s[:, kc, :], Wk_r[:, kc, :])
                nc.sync.dma_start(kT_s[:, kc, :], kT_r[:, kc, :])
                nc.sync.dma_start(Wv_s[:, kc, :], Wv_r[:, kc, :])
                nc.sync.dma_start(vT_s[:, kc, :], vT_r[:, kc, :])

            def proj_q_pair(ec):
                """Emit qhT projection for head-pair ec (interleavable)."""
                ps = psA.tile([128, LQC], F32, tag="ps", name=f"psq_{ec}")
                for kc in range(4):
                    for qc in range(2):
                        nc.tensor.matmul(
                            ps[:, qc * 512:(qc + 1) * 512],
                            lhsT=Wq_s[:, kc, ec * 128:(ec + 1) * 128],
                            rhs=qT_s[:, kc, qc * 512:(qc + 1) * 512],
                            start=(kc == 0), stop=(kc == 3),
                        )
                nc.vector.tensor_scalar_add(qhT_s[:, ec, :], ps, bq_s[:, ec:ec + 1])

            def proj_k_pair(ec):
                """Emit khT projection for head-pair ec (interleavable)."""
                for ci, (s, w) in enumerate(kchunks):
                    ps2 = psA.tile([128, LQC], F32, tag="ps", name=f"psk_{ec}_{ci}")
                    for kc in range(4):
                        nc.tensor.matmul(
                            ps2[:, 0:w],
                            lhsT=Wk_s[:, kc, ec * 128:(ec + 1) * 128],
                            rhs=kT_s[:, kc, s:s + w],
                            start=(kc == 0), stop=(kc == 3),
                        )
                    nc.vector.tensor_scalar_add(
                        khT_s[:, ec, s:s + w], ps2[:, 0:w], bk_s[:, ec:ec + 1]
                    )

            def proj_qk_pair(ec):
                proj_q_pair(ec)
                proj_k_pair(ec)

            # vh[k, e] natural layout in 65-wide head groups (+bv on copy-out)
            for kb in range(NKB):
                ps = psB.tile([128, 512], F32, tag="att", name=f"psv_{kb}")
                for kc in range(4):
                    nc.tensor.matmul(
                        ps,
                        lhsT=vT_s[:, kc, kb * 128:(kb + 1) * 128],
                        rhs=Wv_s[:, kc, :],
                        start=(kc == 0), stop=(kc == 3),
                    )
                nc.vector.scalar_tensor_tensor(
                    vh_s[:, kb, :, 0:64],
                    ps.rearrange("p (h e) -> p h e", h=H),
                    1.0,
                    bv_s.rearrange("p (h e) -> p h e", h=H),
                    op0=ALU.mult, op1=ALU.add,
                )

            # projections for head-pair 0 up front; pairs 1-3 are emitted
            # inside the attention stream so the PE can fill exp-wait gaps
            proj_qk_pair(0)

            # =========== phase 2: attention (inside projin scope for
            # interleaved projections) ===========
            for hp in range(4):
                att_ps = {}
                for hi in range(2):
                    for qc in range(2):
                        att_ps[(hi, qc)] = psB.tile([65, 512], F32, tag="att",
                                                    name=f"att_{hp}_{hi}_{qc}")
                for kb in range(NKB):
                    # interleave next pair's projections mid-stream
                    if kb == 1 and hp < 3:
                        proj_q_pair(hp + 1)
                    if kb == 4 and hp < 3:
                        proj_k_pair(hp + 1)
                    sc = {}
                    for hi in range(2):
                        sc[hi] = psA.tile([128, LQC], F32, tag="ps",
                                          name=f"sc_{hp}_{hi}_{kb}")
                    # alternate row-groups (hi) so score matmuls pair up
                    # concurrently on the PE array
                    for qc in range(2):
                        for hi in range(2):
                            p0 = 64 * hi
                            nc.tensor.matmul(
                                sc[hi][:, qc * 512:(qc + 1) * 512],
                                lhsT=khT_s[p0:p0 + 64, hp,
                                           kb * 128:(kb + 1) * 128],
                                rhs=qhT_s[p0:p0 + 64, hp,
                                          qc * 512:(qc + 1) * 512],
                                start=True, stop=True,
                            )
                    exp_t = {}
                    for hi in range(2):
                        et = work.tile([128, LQC], BF16, tag="exp",
                                       name=f"exp_{hp}_{hi}_{kb}")
                        nc.scalar.activation(et, sc[hi], AF.Exp,
                                             bias=mb_s[:, kb:kb + 1],
                                             scale=SCALE)
                        exp_t[hi] = et
                    for hi in range(2):
                        h = 2 * hp + hi
                        for qc in range(2):
                            nc.tensor.matmul(
                                att_ps[(hi, qc)],
                                lhsT=vh_s[:, kb, h, :],
                                rhs=exp_t[hi][:, qc * 512:(qc + 1) * 512],
                                start=(kb == 0), stop=(kb == NKB - 1),
                            )
                # gather the pair's 4 denominator rows at legal partition
                # bases {0,32,64,96}, one batched reciprocal, one DRAM
                # bounce for the partition broadcast
                dn = work.tile([128, 512], F32, tag="dn", name=f"dn_{hp}")
                nc.vector.memset(dn, 1.0)
                for hi in range(2):
                    for qc in range(2):
                        r = 32 * (2 * hi + qc)
                        nc.vector.tensor_copy(
                            dn[r:r + 1, :], att_ps[(hi, qc)][64:65, :])
                rc4 = work.tile([128, 512], F32, tag="rc4", name=f"rc4_{hp}")
                nc.vector.reciprocal(rc4, dn)
                rc_d = dramp.tile([128, 512], F32, tag="rcd", name=f"rcd_{hp}")
                nc.sync.dma_start(rc_d, rc4)
                for hi in range(2):
                    p0 = 64 * hi
                    for qc in range(2):
                        ap_t = att_ps[(hi, qc)]
                        r = 32 * (2 * hi + qc)
                        rd_ap = rc_d[r:r + 1, :]
                        rc_b = bass.AP(tensor=rd_ap.tensor, offset=rd_ap.offset,
                                       ap=[[0, 64]] + list(rd_ap.ap)[1:])
                        Rt = work.tile([64, 512], F32, tag="R",
                                       name=f"R_{hp}_{hi}_{qc}")
                        nc.sync.dma_start(Rt, rc_b)
                        nc.vector.tensor_tensor(
                            attT_s[p0:p0 + 64, hp, qc * 512:(qc + 1) * 512],
                            ap_t[0:64, :], Rt, op=ALU.mult,
                        )

        # =========== phase 3: out projection + residual + layernorm ===========
        for qb in range(8):
            qres_t = epi.tile([128, D], F32, tag="qres", name=f"qres_{qb}")
            nc.sync.dma_start(
                qres_t, qres_d[qb * 128:(qb + 1) * 128, :])
            po = psB.tile([128, 512], F32, tag="att", name=f"po_{qb}")
            for ec in range(4):
                nc.tensor.matmul(
                    po,
                    lhsT=attT_s[:, ec, qb * 128:(qb + 1) * 128],
                    rhs=Wo_s[:, ec, :],
                    start=(ec == 0), stop=(ec == 3),
                )
            x = epi.tile([128, D], F32, tag="x", name=f"x_{qb}")
            nc.vector.tensor_add(x, po, qres_t)
            st6 = epi.tile([128, 6], F32, tag="st6", name=f"st6_{qb}")
            nc.vector.bn_stats(st6, x)
            mv = epi.tile([128, 2], F32, tag="mv", name=f"mv_{qb}")
            nc.vector.bn_aggr(mv, st6)
            sd = epi.tile([128, 1], F32, tag="sd", name=f"sd_{qb}")
            nc.scalar.activation(sd, mv[:, 1:2], AF.Sqrt, bias=eps_s[:, 0:1],
                                 scale=1.0)
            rs = epi.tile([128, 1], F32, tag="rs", name=f"rs_{qb}")
            nc.vector.reciprocal(rs, sd)
            # -mu * rstd so the LN affine fits one ScalarE activation
            nmu = epi.tile([128, 1], F32, tag="nmu", name=f"nmu_{qb}")
            nc.vector.scalar_tensor_tensor(nmu, mv[:, 0:1], -1.0, rs,
                                           op0=ALU.mult, op1=ALU.mult)
            y = epi.tile([128, D], F32, tag="y", name=f"y_{qb}")
            nc.scalar.activation(y, x, AF.Identity,
                                 bias=nmu[:, 0:1], scale=rs[:, 0:1])
            # gamma/beta on GpSimd: keeps the tail off the busy DVE
            y2 = epi.tile([128, D], F32, tag="qres", name=f"y2_{qb}")
            nc.gpsimd.tensor_tensor(y2, y, gamma_s, op=ALU.mult)
            y3 = epi.tile([128, D], F32, tag="x", name=f"y3_{qb}")
            nc.gpsimd.tensor_tensor(y3, y2, beta_s, op=ALU.add)
            nc.sync.dma_start(out_d[qb * 128:(qb + 1) * 128, :], y3)

    return nc


def prepare_host_inputs(q, k, v, att_mask, Wq, bq, Wk, bk, Wv, bv, Wo, bo,
                        gamma, beta):
    """Host-side sharding: mask compaction, transposes, per-core in_maps."""
    q = np.asarray(q, np.float32)
    k = np.asarray(k, np.float32)
    v = np.asarray(v, np.float32)
    att_mask = np.asarray(att_mask)
    valid = ~att_mask  # True = keep
    counts = valid.sum(axis=1)
    LKP = int(max(128, ((int(counts.max()) + 127) // 128) * 128))
    NKB = LKP // 128

    kT_p = np.zeros((B, D, LKP), NPBF)
    vT_p = np.zeros((B, D, LKP), NPBF)
    mb = np.full((B, LKP), MASK_NEG, np.float32)
    for b in range(B):
        idx = np.nonzero(valid[b])[0]
        n = len(idx)
        kT_p[b, :, :n] = k[b, idx, :].T.astype(NPBF)
        vT_p[b, :, :n] = v[b, idx, :].T.astype(NPBF)
        mb[b, :n] = 0.0

    shared = {
        "WqT": np.ascontiguousarray(np.asarray(Wq, np.float32).T).astype(NPBF),
        "WkT": np.ascontiguousarray(np.asarray(Wk, np.float32).T).astype(NPBF),
        "WvT": np.ascontiguousarray(np.asarray(Wv, np.float32).T).astype(NPBF),
        "WoT": np.ascontiguousarray(np.asarray(Wo, np.float32).T).astype(NPBF),
        "bq": np.ascontiguousarray(np.asarray(bq, np.float32).reshape(4, 128).T),
        "bk": np.ascontiguousarray(np.asarray(bk, np.float32).reshape(4, 128).T),
        "bv": np.asarray(bv, np.float32).copy(),
        "gamma": np.asarray(gamma, np.float32).copy(),
        "beta": np.asarray(beta, np.float32).copy(),
    }
    bo_f = np.asarray(bo, np.float32)

    in_maps = []
    for core in range(NCORES):
        b, qh = divmod(core, 2)
        sl = slice(qh * LQC, (qh + 1) * LQC)
        m = dict(shared)
        m["qT"] = np.ascontiguousarray(q[b, sl, :].T).astype(NPBF)
        m["qres"] = q[b, sl, :] + bo_f  # residual with out-proj bias folded in
        m["kT"] = kT_p[b]
        m["vT"] = vT_p[b]
        m["mb"] = np.ascontiguousarray(mb[b].reshape(NKB, 128).T)
        in_maps.append(m)
    return in_maps, LKP


_program_cache = {}


def get_program(LKP):
    if LKP not in _program_cache:
        nc = bacc.Bacc("TRN2", target_bir_lowering=False, debug=False)
        build_core_program(nc, LKP)
        nc.compile()
        _program_cache[LKP] = nc
    return _program_cache[LKP]


def kernel(q, k, v, att_mask, Wq, bq, Wk, bk, Wv, bv, Wo, bo, gamma, beta,
           _trace=False, _trace_kwargs=None):
    in_maps, LKP = prepare_host_inputs(
        q, k, v, att_mask, Wq, bq, Wk, bk, Wv, bv, Wo, bo, gamma, beta)
    nc = get_program(LKP)
    res = run_bass_kernel_spmd(
        nc, in_maps, core_ids=list(range(NCORES)),
        trace=_trace, **(_trace_kwargs or {}),
    )
    out = np.empty((B, LQ, D), np.float32)
    for core in range(NCORES):
        b, qh = divmod(core, 2)
        out[b, qh * LQC:(qh + 1) * LQC, :] = res.results[core]["out"]
    kernel._last_results = res
    return out


# revision 39
# speedup vs baseline: 1.4072x; 1.1526x over previous
"""Multi-head attention + residual + layernorm Trainium2 kernel.

Sharding: 8 cores = 4 batches x 2 query-halves (data parallel, no
collectives). Each core computes full MHA for 1024 query rows of one
batch against that batch's (mask-compacted) keys/values.

Device pipeline per core:
  - QKV projections (bf16 matmuls, fp32 PSUM accumulation)
  - per head: scoresT[keys, queries] = khT.T @ qhT
  - exp fused with mask bias + 1/sqrt(dk) scale on ScalarE
  - attT accumulation with ones-column -> softmax denominators for free
  - normalize via reciprocal + partition-broadcast
  - out projection, residual add (fp32 q), LayerNorm (bn_stats/bn_aggr)
"""

import math
from contextlib import ExitStack

import ml_dtypes
import numpy as np

import concourse.bass as bass
import concourse.mybir as mybir
import concourse.tile as tile
from concourse import bacc
from concourse.bass_utils import run_bass_kernel_spmd

# problem constants (hardcoded per harness contract)
B = 4
LQ = 2048
LK = 2048
D = 512
H = 8
DK = 64
NCORES = 8
LQC = LQ // 2  # queries per core
EPS = 1e-6
SCALE = 1.0 / math.sqrt(DK)
MASK_NEG = -30000.0

F32 = mybir.dt.float32
BF16 = mybir.dt.bfloat16
NPBF = ml_dtypes.bfloat16

AF = mybir.ActivationFunctionType
ALU = mybir.AluOpType


def build_core_program(nc, LKP, gb_identity=False):
    """Emit the single-core SPMD program. LKP = padded compacted key count.
    gb_identity: gamma==1 and beta==0, so the LN affine tail can be skipped."""
    NKB = LKP // 128
    kchunks = [(s, min(512, LKP - s)) for s in range(0, LKP, 512)]

    # ---- DRAM parameters (per-core shards supplied via in_maps) ----
    qT_d = nc.declare_dram_parameter("qT", [D, LQC], BF16, isOutput=False)
    qres_d = nc.declare_dram_parameter("qres", [LQC, D], F32, isOutput=False)
    kT_d = nc.declare_dram_parameter("kT", [D, LKP], BF16, isOutput=False)
    vT_d = nc.declare_dram_parameter("vT", [D, LKP], BF16, isOutput=False)
    mb_d = nc.declare_dram_parameter("mb", [128, NKB], F32, isOutput=False)
    WqT_d = nc.declare_dram_parameter("WqT", [D, D], BF16, isOutput=False)
    WkT_d = nc.declare_dram_parameter("WkT", [D, D], BF16, isOutput=False)
    WvT_d = nc.declare_dram_parameter("WvT", [D, D], BF16, isOutput=False)
    WoT_d = nc.declare_dram_parameter("WoT", [D, D], BF16, isOutput=False)
    bq_d = nc.declare_dram_parameter("bq", [128, 4], F32, isOutput=False)
    bk_d = nc.declare_dram_parameter("bk", [128, 4], F32, isOutput=False)
    bv_d = nc.declare_dram_parameter("bv", [D], F32, isOutput=False)
    gamma_d = nc.declare_dram_parameter("gamma", [D], F32, isOutput=False)
    beta_d = nc.declare_dram_parameter("beta", [D], F32, isOutput=False)
    out_d = nc.declare_dram_parameter("out", [LQC, D], F32, isOutput=True)

    with ExitStack() as ctx:
        tc = ctx.enter_context(tile.TileContext(nc))

        const = ctx.enter_context(tc.tile_pool(name="const", bufs=1))
        persist = ctx.enter_context(tc.tile_pool(name="persist", bufs=1))
        work = ctx.enter_context(tc.tile_pool(name="work", bufs=3))
        epi = ctx.enter_context(tc.tile_pool(name="epi", bufs=2))
        psA = ctx.enter_context(tc.tile_pool(name="psA", bufs=2, space="PSUM"))
        psB = ctx.enter_context(tc.tile_pool(name="psB", bufs=4, space="PSUM"))
        dramp = ctx.enter_context(tc.tile_pool(name="dramp", bufs=4, space="DRAM"))

        # ---- constants ----
        eps_s = const.tile([128, 1], F32)
        nc.vector.memset(eps_s, EPS)
        mb_s = const.tile([128, NKB], F32)
        nc.sync.dma_start(mb_s, mb_d[:, :])
        bq_s = const.tile([128, 4], F32)
        nc.sync.dma_start(bq_s, bq_d[:, :])
        bk_s = const.tile([128, 4], F32)
        nc.sync.dma_start(bk_s, bk_d[:, :])

        def _pbcast(handle):
            ap = handle.ap()
            return bass.AP(tensor=ap.tensor, offset=ap.offset,
                           ap=[[0, 128]] + list(ap.ap))

        bv_s = const.tile([128, D], F32)
        nc.sync.dma_start(bv_s, _pbcast(bv_d))
        gamma_s = const.tile([128, D], F32)
        nc.sync.dma_start(gamma_s, _pbcast(gamma_d))
        beta_s = const.tile([128, D], F32)
        nc.sync.dma_start(beta_s, _pbcast(beta_d))

        # ---- persistent intermediates ----
        qhT_s = persist.tile([128, 4, LQC], BF16)    # [e%128, e//128, q]
        khT_s = persist.tile([128, 4, LKP], BF16)    # [e%128, e//128, k]
        vh_s = persist.tile([128, NKB, H, 65], BF16)  # [k%128, kb, h, dv+1]
        attT_s = persist.tile([128, 4, LQC], BF16)   # [e%128, e//128, q]
        Wo_s = persist.tile([128, 4, D], BF16)       # [e%128, e//128, d]

        nc.sync.dma_start(Wo_s, WoT_d.ap().rearrange("(c p) d -> p c d", p=128))

        # ones column of vh (softmax denominator trick)
        nc.vector.memset(vh_s[:, :, :, 64:65], 1.0)

        # =========== phase 1: projections ===========
        with tc.tile_pool(name="projin", bufs=1) as projin:
            # per-kc chunk DMAs so the first matmuls start as soon as their
            # operands land instead of after the whole input batch
            qT_s = projin.tile([128, 4, LQC], BF16)
            kT_s = projin.tile([128, 4, LKP], BF16)
            vT_s = projin.tile([128, 4, LKP], BF16)
            Wq_s = projin.tile([128, 4, D], BF16)
            Wk_s = projin.tile([128, 4, D], BF16)
            Wv_s = projin.tile([128, 4, D], BF16)
            qT_r = qT_d.ap().rearrange("(c p) q -> p c q", p=128)
            kT_r = kT_d.ap().rearrange("(c p) k -> p c k", p=128)
            vT_r = vT_d.ap().rearrange("(c p) k -> p c k", p=128)
            Wq_r = WqT_d.ap().rearrange("(c p) e -> p c e", p=128)
            Wk_r = WkT_d.ap().rearrange("(c p) e -> p c e", p=128)
            Wv_r = WvT_d.ap().rearrange("(c p) e -> p c e", p=128)
            # priority order: q-projection operands first, then k, then v
            for kc in range(4):
                nc.sync.dma_start(Wq_s[:, kc, :], Wq_r[:, kc, :])
                nc.sync.dma_start(qT_s[:, kc, :], qT_r[:, kc, :])
            for kc in range(4):
                nc.sync.dma_start(Wk_s[:, kc, :], Wk_r[:, kc, :])
                nc.sync.dma_start(kT_s[:, kc, :], kT_r[:, kc, :])
            for kc in range(4):
                nc.sync.dma_start(Wv_s[:, kc, :], Wv_r[:, kc, :])
                nc.sync.dma_start(vT_s[:, kc, :], vT_r[:, kc, :])

            def proj_q_pair(ec):
                """Emit qhT projection for head-pair ec (interleavable)."""
                ps = psA.tile([128, LQC], F32, tag="ps", name=f"psq_{ec}")
                for kc in range(4):
                    for qc in range(2):
                        nc.tensor.matmul(
                            ps[:, qc * 512:(qc + 1) * 512],
                            lhsT=Wq_s[:, kc, ec * 128:(ec + 1) * 128],
                            rhs=qT_s[:, kc, qc * 512:(qc + 1) * 512],
                            start=(kc == 0), stop=(kc == 3),
                        )
                nc.vector.tensor_scalar_add(qhT_s[:, ec, :], ps, bq_s[:, ec:ec + 1])

            def proj_k_pair(ec):
                """Emit khT projection for head-pair ec (interleavable)."""
                for ci, (s, w) in enumerate(kchunks):
                    ps2 = psA.tile([128, LQC], F32, tag="ps", name=f"psk_{ec}_{ci}")
                    for kc in range(4):
                        nc.tensor.matmul(
                            ps2[:, 0:w],
                            lhsT=Wk_s[:, kc, ec * 128:(ec + 1) * 128],
                            rhs=kT_s[:, kc, s:s + w],
                            start=(kc == 0), stop=(kc == 3),
                        )
                    nc.vector.tensor_scalar_add(
                        khT_s[:, ec, s:s + w], ps2[:, 0:w], bk_s[:, ec:ec + 1]
                    )

            def proj_qk_pair(ec):
                proj_q_pair(ec)
                proj_k_pair(ec)

            def proj_v(kb):
                """vh[k, e] natural layout in 65-wide head groups (+bv)."""
                ps = psA.tile([128, 512], F32, tag="ps", name=f"psv_{kb}")
                for kc in range(4):
                    nc.tensor.matmul(
                        ps,
                        lhsT=vT_s[:, kc, kb * 128:(kb + 1) * 128],
                        rhs=Wv_s[:, kc, :],
                        start=(kc == 0), stop=(kc == 3),
                    )
                nc.vector.scalar_tensor_tensor(
                    vh_s[:, kb, :, 0:64],
                    ps.rearrange("p (h e) -> p h e", h=H),
                    1.0,
                    bv_s.rearrange("p (h e) -> p h e", h=H),
                    op0=ALU.mult, op1=ALU.add,
                )

            # q/k projections for head-pair 0 first (unblocks ScalarE exps
            # ASAP), first two vh blocks, rest of vh interleaved into pair 0
            proj_qk_pair(0)
            proj_v(0)
            proj_v(1)

            # =========== phase 2: attention (inside projin scope for
            # interleaved projections) ===========
            for hp in range(4):
                att_ps = {}
                for hi in range(2):
                    for qc in range(2):
                        att_ps[(hi, qc)] = psB.tile([65, 512], F32, tag="att",
                                                    name=f"att_{hp}_{hi}_{qc}")
                for kb in range(NKB):
                    # pair 0 also streams the remaining vh projections
                    if hp == 0 and kb + 2 < NKB:
                        proj_v(kb + 2)
                    # interleave next pair's projections mid-stream
                    if kb == 1 and hp < 3:
                        proj_q_pair(hp + 1)
                    if kb == 4 and hp < 3:
                        proj_k_pair(hp + 1)
                    sc = {}
                    for hi in range(2):
                        sc[hi] = psA.tile([128, LQC], F32, tag="ps",
                                          name=f"sc_{hp}_{hi}_{kb}")
                    # alternate row-groups (hi) so score matmuls pair up
                    # concurrently on the PE array
                    for qc in range(2):
                        for hi in range(2):
                            p0 = 64 * hi
                            nc.tensor.matmul(
                                sc[hi][:, qc * 512:(qc + 1) * 512],
                                lhsT=khT_s[p0:p0 + 64, hp,
                                           kb * 128:(kb + 1) * 128],
                                rhs=qhT_s[p0:p0 + 64, hp,
                                          qc * 512:(qc + 1) * 512],
                                start=True, stop=True,
                            )
                    exp_t = {}
                    for hi in range(2):
                        et = work.tile([128, LQC], BF16, tag="exp",
                                       name=f"exp_{hp}_{hi}_{kb}")
                        nc.scalar.activation(et, sc[hi], AF.Exp,
                                             bias=mb_s[:, kb:kb + 1],
                                             scale=SCALE)
                        exp_t[hi] = et
                    for hi in range(2):
                        h = 2 * hp + hi
                        for qc in range(2):
                            nc.tensor.matmul(
                                att_ps[(hi, qc)],
                                lhsT=vh_s[:, kb, h, :],
                                rhs=exp_t[hi][:, qc * 512:(qc + 1) * 512],
                                start=(kb == 0), stop=(kb == NKB - 1),
                            )
                # copy raw att psum out immediately -> frees the PSUM banks
                # for the next pair; normalization then runs off-critical-path
                # from SBUF on the (otherwise idle) GpSimd engine
                araw = {}
                for hi in range(2):
                    for qc in range(2):
                        at = work.tile([65, 512], F32, tag="araw", bufs=5,
                                       name=f"araw_{hp}_{hi}_{qc}")
                        nc.vector.tensor_copy(at, att_ps[(hi, qc)])
                        araw[(hi, qc)] = at
                # gather the pair's 4 denominator rows at legal partition
                # bases {0,32,64,96}, one batched reciprocal, one DRAM
                # bounce for the partition broadcast
                dn = work.tile([128, 512], F32, tag="dn", name=f"dn_{hp}")
                nc.gpsimd.memset(dn, 1.0)
                for hi in range(2):
                    for qc in range(2):
                        r = 32 * (2 * hi + qc)
                        nc.gpsimd.tensor_copy(
                            dn[r:r + 1, :], araw[(hi, qc)][64:65, :])
                rc4 = work.tile([128, 512], F32, tag="rc4", name=f"rc4_{hp}")
                nc.vector.reciprocal(rc4, dn)
                rc_d = dramp.tile([128, 512], F32, tag="rcd", name=f"rcd_{hp}")
                nc.sync.dma_start(rc_d, rc4)
                for hi in range(2):
                    p0 = 64 * hi
                    for qc in range(2):
                        r = 32 * (2 * hi + qc)
                        rd_ap = rc_d[r:r + 1, :]
                        rc_b = bass.AP(tensor=rd_ap.tensor, offset=rd_ap.offset,
                                       ap=[[0, 64]] + list(rd_ap.ap)[1:])
                        Rt = work.tile([64, 512], F32, tag="R",
                                       name=f"R_{hp}_{hi}_{qc}")
                        nc.sync.dma_start(Rt, rc_b)
                        nc.gpsimd.tensor_tensor(
                            attT_s[p0:p0 + 64, hp, qc * 512:(qc + 1) * 512],
                            araw[(hi, qc)][0:64, :], Rt, op=ALU.mult,
                        )

        # =========== phase 3: out projection + residual + layernorm ===========
        for qb in range(8):
            qres_t = epi.tile([128, D], F32, tag="qres", name=f"qres_{qb}")
            nc.sync.dma_start(
                qres_t, qres_d[qb * 128:(qb + 1) * 128, :])
            po = psB.tile([128, 512], F32, tag="att", name=f"po_{qb}")
            for ec in range(4):
                nc.tensor.matmul(
                    po,
                    lhsT=attT_s[:, ec, qb * 128:(qb + 1) * 128],
                    rhs=Wo_s[:, ec, :],
                    start=(ec == 0), stop=(ec == 3),
                )
            x = epi.tile([128, D], F32, tag="x", name=f"x_{qb}")
            nc.vector.tensor_add(x, po, qres_t)
            st6 = epi.tile([128, 6], F32, tag="st6", name=f"st6_{qb}")
            nc.vector.bn_stats(st6, x)
            mv = epi.tile([128, 2], F32, tag="mv", name=f"mv_{qb}")
            nc.vector.bn_aggr(mv, st6)
            sd = epi.tile([128, 1], F32, tag="sd", name=f"sd_{qb}")
            nc.scalar.activation(sd, mv[:, 1:2], AF.Sqrt, bias=eps_s[:, 0:1],
                                 scale=1.0)
            rs = epi.tile([128, 1], F32, tag="rs", name=f"rs_{qb}")
            nc.vector.reciprocal(rs, sd)
            # -mu * rstd so the LN affine fits one ScalarE activation
            nmu = epi.tile([128, 1], F32, tag="nmu", name=f"nmu_{qb}")
            nc.vector.scalar_tensor_tensor(nmu, mv[:, 0:1], -1.0, rs,
                                           op0=ALU.mult, op1=ALU.mult)
            y = epi.tile([128, D], F32, tag="y", name=f"y_{qb}")
            nc.scalar.activation(y, x, AF.Identity,
                                 bias=nmu[:, 0:1], scale=rs[:, 0:1])
            if gb_identity:
                nc.sync.dma_start(out_d[qb * 128:(qb + 1) * 128, :], y)
            else:
                # gamma/beta on GpSimd: keeps the tail off the busy DVE
                y2 = epi.tile([128, D], F32, tag="qres", name=f"y2_{qb}")
                nc.gpsimd.tensor_tensor(y2, y, gamma_s, op=ALU.mult)
                y3 = epi.tile([128, D], F32, tag="x", name=f"y3_{qb}")
                nc.gpsimd.tensor_tensor(y3, y2, beta_s, op=ALU.add)
                nc.sync.dma_start(out_d[qb * 128:(qb + 1) * 128, :], y3)

    return nc


def prepare_host_inputs(q, k, v, att_mask, Wq, bq, Wk, bk, Wv, bv, Wo, bo,
                        gamma, beta):
    """Host-side sharding: mask compaction, transposes, per-core in_maps."""
    q = np.asarray(q, np.float32)
    k = np.asarray(k, np.float32)
    v = np.asarray(v, np.float32)
    att_mask = np.asarray(att_mask)
    valid = ~att_mask  # True = keep
    counts = valid.sum(axis=1)
    LKP = int(max(128, ((int(counts.max()) + 127) // 128) * 128))
    NKB = LKP // 128

    kT_p = np.zeros((B, D, LKP), NPBF)
    vT_p = np.zeros((B, D, LKP), NPBF)
    mb = np.full((B, LKP), MASK_NEG, np.float32)
    for b in range(B):
        idx = np.nonzero(valid[b])[0]
        n = len(idx)
        kT_p[b, :, :n] = k[b, idx, :].T.astype(NPBF)
        vT_p[b, :, :n] = v[b, idx, :].T.astype(NPBF)
        mb[b, :n] = 0.0

    shared = {
        "WqT": np.ascontiguousarray(np.asarray(Wq, np.float32).T).astype(NPBF),
        "WkT": np.ascontiguousarray(np.asarray(Wk, np.float32).T).astype(NPBF),
        "WvT": np.ascontiguousarray(np.asarray(Wv, np.float32).T).astype(NPBF),
        "WoT": np.ascontiguousarray(np.asarray(Wo, np.float32).T).astype(NPBF),
        "bq": np.ascontiguousarray(np.asarray(bq, np.float32).reshape(4, 128).T),
        "bk": np.ascontiguousarray(np.asarray(bk, np.float32).reshape(4, 128).T),
        "bv": np.asarray(bv, np.float32).copy(),
        "gamma": np.asarray(gamma, np.float32).copy(),
        "beta": np.asarray(beta, np.float32).copy(),
    }
    bo_f = np.asarray(bo, np.float32)

    in_maps = []
    for core in range(NCORES):
        b, qh = divmod(core, 2)
        sl = slice(qh * LQC, (qh + 1) * LQC)
        m = dict(shared)
        m["qT"] = np.ascontiguousarray(q[b, sl, :].T).astype(NPBF)
        m["qres"] = q[b, sl, :] + bo_f  # residual with out-proj bias folded in
        m["kT"] = kT_p[b]
        m["vT"] = vT_p[b]
        m["mb"] = np.ascontiguousarray(mb[b].reshape(NKB, 128).T)
        in_maps.append(m)
    return in_maps, LKP


_program_cache = {}


def get_program(LKP, gb_identity=False):
    key = (LKP, gb_identity)
    if key not in _program_cache:
        nc = bacc.Bacc("TRN2", target_bir_lowering=False, debug=False)
        build_core_program(nc, LKP, gb_identity=gb_identity)
        nc.compile()
        _program_cache[key] = nc
    return _program_cache[key]


def kernel(q, k, v, att_mask, Wq, bq, Wk, bk, Wv, bv, Wo, bo, gamma, beta,
           _trace=False, _trace_kwargs=None):
    in_maps, LKP = prepare_host_inputs(
        q, k, v, att_mask, Wq, bq, Wk, bk, Wv, bv, Wo, bo, gamma, beta)
    gb_identity = bool(np.all(np.asarray(gamma) == 1.0)
                       and np.all(np.asarray(beta) == 0.0))
    nc = get_program(LKP, gb_identity)
    res = run_bass_kernel_spmd(
        nc, in_maps, core_ids=list(range(NCORES)),
        trace=_trace, **(_trace_kwargs or {}),
    )
    out = np.empty((B, LQ, D), np.float32)
    for core in range(NCORES):
        b, qh = divmod(core, 2)
        out[b, qh * LQC:(qh + 1) * LQC, :] = res.results[core]["out"]
    kernel._last_results = res
    return out
